# revision 10
# baseline (speedup 1.0000x reference)
"""Trainium2 Bass kernel for a transformer decoder layer (self-attn + cross-attn + FFN).

Sharding: 8 cores = 4 batches x 2 query-halves. Zero collectives: each core
computes the full layer for its 512 query tokens (feature-major layout), and
recomputes the full-sequence K/V for its batch (cheap relative to total work).
Causality is handled purely by mask data: each core sees K/V token chunks in
core-local order (own half first), so one SPMD program serves all cores.

All heavy matmuls run as float32r (full PE rate, ~1e-4 relative error).
Softmax: no max-subtraction (scores are O(6); exp is safe in fp32); the
denominator comes free from a ones-column appended to V in the PV matmul.
"""
import sys
sys.path.insert(0, '/opt/trn_rl_repo')

import numpy as np

import concourse.bass as bass
from concourse import bacc
import concourse.mybir as mybir
import concourse.tile as tile
from concourse.bass_utils import run_bass_kernel_spmd
from concourse.masks import make_identity
from contextlib import ExitStack

F32 = mybir.dt.float32
F32R = mybir.dt.float32r
Act = mybir.ActivationFunctionType
Alu = mybir.AluOpType

B, LT, LS, D, H, HD, F = 4, 1024, 1024, 1024, 16, 64, 4096
TQ = LT // 2          # tokens per core (query half)
NB = D // 128         # feature blocks (8)
NKB = LT // 128       # key blocks (8)
NFB = F // 128        # FFN feature blocks (32)
EPS = 1e-6


def _build():
    nc = bacc.Bacc("TRN2", target_bir_lowering=False)

    di = lambda n, shp: nc.dram_tensor(n, shp, F32, kind="ExternalInput")
    do = lambda n, shp: nc.dram_tensor(n, shp, F32, kind="ExternalOutput")

    xq_d = di("xq", [D, TQ])            # own x chunk, feature-major
    xo_d = di("xo", [D, TQ])            # other x chunk
    mq_d = di("mq", [D, TQ])            # own memory chunk
    mo_d = di("mo", [D, TQ])            # other memory chunk
    maskT_d = di("maskT", [LT, TQ])     # causal mask [k(core order), q]
    peT_d = di("peT", [H, LS, TQ])      # position bias [h, k(core order), q]
    g_sa_d = di("g_sa", [128, NB])
    g_ca_d = di("g_ca", [128, NB])
    g_m_d = di("g_m", [128, NB])
    w_d = {n: di(n, [D, D]) for n in
           ("wq_s", "wk_s", "wv_s", "wo_s", "wq_c", "wk_c", "wv_c", "wo_c")}
    w1_d = di("w1", [D, F])
    w2_d = di("w2", [F, D])

    mlp_d = do("mlpT", [D, TQ])
    ks_d = do("ks_own", [D, TQ])
    vs_d = do("vs_own", [TQ, D])
    kc_d = do("kc_own", [D, TQ])
    vc_d = do("vc_own", [TQ, D])

    def w_slab_ap(w, lo, n):
        # [D, D] -> [128, NB, n] slice of columns [lo, lo+n)
        return w.ap().rearrange("(o p) d -> p o d", p=128)[:, :, lo:lo + n].bitcast(F32R)

    with tile.TileContext(nc) as tc, ExitStack() as top:
        const = top.enter_context(tc.tile_pool(name="const", bufs=1))
        tiny = top.enter_context(tc.tile_pool(name="tiny", bufs=2))
        wpool = top.enter_context(tc.tile_pool(name="wpool", bufs=2))
        pspool = top.enter_context(tc.tile_pool(name="pspool", bufs=2, space="PSUM"))
        sqpool = top.enter_context(tc.tile_pool(name="sqpool", bufs=2))
        norm_ps = top.enter_context(tc.tile_pool(name="norm_ps", bufs=1, space="PSUM"))
        bcpool = top.enter_context(tc.tile_pool(name="bcpool", bufs=2))
        axp = top.enter_context(tc.tile_pool(name="axp", bufs=1))

        ident_f = const.tile([128, 128], F32)
        make_identity(nc, ident_f[:])
        ident = const.tile([128, 128], F32R)
        nc.vector.tensor_copy(out=ident[:], in_=ident_f[:])
        ones_f = const.tile([128, 1], F32)
        nc.vector.memset(ones_f[:], 1.0)
        ones_r = const.tile([128, 1], F32R)
        nc.vector.tensor_copy(out=ones_r[:], in_=ones_f[:])
        eps_t = const.tile([1, 1], F32)
        nc.vector.memset(eps_t[:], EPS)
        g_sa = const.tile([128, NB], F32)
        g_ca = const.tile([128, NB], F32)
        g_m = const.tile([128, NB], F32)
        nc.sync.dma_start(g_sa[:], g_sa_d.ap())
        nc.sync.dma_start(g_ca[:], g_ca_d.ap())
        nc.sync.dma_start(g_m[:], g_m_d.ap())

        def rmsnorm(x_t, gamma, out_t):
            """x_t [128, NB, TQ] F32 -> out_t [128, NB, TQ] F32R (x * rstd * gamma)."""
            ps = norm_ps.tile([1, TQ], F32, tag="norm_ps")
            for b in range(NB):
                sq = sqpool.tile([128, TQ], F32R, tag="sq")
                nc.vector.tensor_mul(out=sq[:], in0=x_t[:, b, :], in1=x_t[:, b, :])
                nc.tensor.matmul(ps[:], ones_r[:], sq[:],
                                 start=(b == 0), stop=(b == NB - 1))
            rstd = tiny.tile([1, TQ], F32, tag="rstd")
            nc.scalar.activation(out=rstd[:], in_=ps[:], func=Act.Sqrt,
                                 bias=eps_t[:], scale=1.0 / D)
            nc.vector.reciprocal(out=rstd[:], in_=rstd[:])
            rstd_b = tiny.tile([128, TQ], F32, tag="rstd_b")
            nc.gpsimd.partition_broadcast(rstd_b[:], rstd[:])
            for b in range(NB):
                nc.vector.scalar_tensor_tensor(
                    out=out_t[:, b, :], in0=x_t[:, b, :], scalar=gamma[:, b:b + 1],
                    in1=rstd_b[:], op0=Alu.mult, op1=Alu.mult)

        def project(w, rhs_list, consumer):
            """out[dout, t] = sum_din w[din, dout] * rhs[din, t].
            rhs_list: list of [128, NB, TQ] F32R tiles (column chunks).
            consumer(dout_blk, chunk_idx, psum[128, TQ])."""
            for so in range(2):
                w_sb = wpool.tile([128, NB, 512], F32R, tag="wslab")
                nc.sync.dma_start(w_sb[:], w_slab_ap(w, so * 512, 512))
                for d4 in range(4):
                    dout = so * 4 + d4
                    for ci, rhs in enumerate(rhs_list):
                        ps = pspool.tile([128, TQ], F32, tag="proj_ps")
                        for din in range(NB):
                            nc.tensor.matmul(
                                ps[:], w_sb[:, din, d4 * 128:(d4 + 1) * 128],
                                rhs[:, din, :],
                                start=(din == 0), stop=(din == NB - 1))
                        consumer(dout, ci, ps)

        def project_tokmajor(w, rhs_list, v_aug):
            """v[t, dout] token-major into v_aug [128, NKB, H*65] (65-col head groups)."""
            for half in range(2):
                w_sb = wpool.tile([128, NB, 512], F32R, tag="wslab")
                nc.sync.dma_start(w_sb[:], w_slab_ap(w, half * 512, 512))
                for ci, rhs in enumerate(rhs_list):
                    for tb in range(4):
                        ps = pspool.tile([128, TQ], F32, tag="proj_ps")
                        for din in range(NB):
                            nc.tensor.matmul(
                                ps[:], rhs[:, din, tb * 128:(tb + 1) * 128],
                                w_sb[:, din, :],
                                start=(din == 0), stop=(din == NB - 1))
                        dst = v_aug[:, ci * 4 + tb, :].rearrange(
                            "p (h x) -> p h x", x=65)[:, half * 8:(half + 1) * 8, 0:64]
                        nc.scalar.copy(out=dst, in_=ps[:].rearrange("p (h hd) -> p h hd", hd=64))

        def attention(kT, v_aug, qT, att_ps, att_pso, ppool, get_bias, oT_r):
            """Attention with fused per-head softmax normalization -> oT_r [128, NB, TQ] F32R."""
            for h in range(H):
                po = 64 * (h % 2)
                db = h // 2
                psum_o = att_pso.tile([65, TQ], F32, tag="psum_o")
                for kb in range(NKB):
                    psum_s = att_ps.tile([128, TQ], F32, tag="psum_s")
                    nc.tensor.matmul(psum_s[:],
                                     kT[po:po + 64, db, kb * 128:(kb + 1) * 128],
                                     qT[po:po + 64, db, :], start=True, stop=False)
                    nc.tensor.matmul(psum_s[:], ident[:], get_bias(h, kb),
                                     start=False, stop=True)
                    p_sb = ppool.tile([128, TQ], F32R, tag="p")
                    nc.scalar.activation(out=p_sb[:], in_=psum_s[:], func=Act.Exp)
                    nc.tensor.matmul(psum_o[:], v_aug[:, kb, h * 65:(h + 1) * 65],
                                     p_sb[:], start=(kb == 0), stop=(kb == NKB - 1))
                rsum = tiny.tile([1, TQ], F32, tag="rsum")
                nc.vector.reciprocal(out=rsum[:], in_=psum_o[64:65, :])
                bc = bcpool.tile([128, TQ], F32, tag="bc")
                nc.gpsimd.partition_broadcast(bc[:], rsum[:])
                nc.vector.tensor_mul(out=oT_r[po:po + 64, db, :],
                                     in0=psum_o[0:64, :], in1=bc[0:64, :])

        ax = axp.tile([128, NB, TQ], F32)

        # ================= Phases A-C: self-attention block =================
        with ExitStack() as abc:
            xq = abc.enter_context(tc.tile_pool(name="xqp", bufs=1)).tile([128, NB, TQ], F32, name="xq")
            nc.sync.dma_start(xq[:], xq_d.ap().rearrange("(o p) t -> p o t", p=128))
            with ExitStack() as att_scope:
                with ExitStack() as hs:
                    hp = hs.enter_context(tc.tile_pool(name="hpool", bufs=1))
                    h0 = hp.tile([128, NB, TQ], F32R, tag="h0")
                    h1 = hp.tile([128, NB, TQ], F32R, tag="h1")
                    with tc.tile_pool(name="xop", bufs=1) as xop:
                        xo = xop.tile([128, NB, TQ], F32)
                        nc.sync.dma_start(xo[:], xo_d.ap().rearrange("(o p) t -> p o t", p=128))
                        rmsnorm(xq, g_sa, h0)
                        rmsnorm(xo, g_sa, h1)
                    atti = att_scope.enter_context(tc.tile_pool(name="att_in", bufs=1, side="right"))
                    qT = atti.tile([128, NB, TQ], F32R, tag="qT")
                    kT = atti.tile([128, NB, LT], F32R, tag="kT")
                    v_aug = atti.tile([128, NKB, H * 65], F32R, tag="v_aug")
                    nc.vector.memset(
                        v_aug[:].rearrange("p kb (h x) -> p kb h x", x=65)[:, :, :, 64:65].bitcast(F32), 1.0)
                    project(w_d["wq_s"], [h0],
                            lambda dout, ci, ps: nc.scalar.mul(out=qT[:, dout, :], in_=ps[:], mul=0.125))
                    project(w_d["wk_s"], [h0, h1],
                            lambda dout, ci, ps: nc.scalar.copy(
                                out=kT[:, dout, ci * TQ:(ci + 1) * TQ], in_=ps[:]))
                    project_tokmajor(w_d["wv_s"], [h0, h1], v_aug)

                # K/V outputs (own chunk = chunk 0)
                nc.sync.dma_start(ks_d.ap().rearrange("(o p) t -> p o t", p=128),
                                  kT[:, :, 0:TQ].bitcast(F32))
                for tb in range(4):
                    nc.sync.dma_start(
                        vs_d.ap()[tb * 128:(tb + 1) * 128, :].rearrange(
                            "p (h hd) -> p h hd", hd=64),
                        v_aug[:, tb, :].rearrange("p (h x) -> p h x", x=65)[:, :, 0:64].bitcast(F32))

                oT_r = abc.enter_context(tc.tile_pool(name="oTrp", bufs=1)).tile(
                    [128, NB, TQ], F32R, name="oT_r")
                with tc.tile_pool(name="maskp", bufs=1) as maskp, \
                     tc.tile_pool(name="att_ps", bufs=3, space="PSUM") as att_ps, \
                     tc.tile_pool(name="att_pso", bufs=2, space="PSUM") as att_pso, \
                     tc.tile_pool(name="ppool", bufs=2) as ppool:
                    maskT = maskp.tile([128, NKB, TQ], F32R)
                    nc.sync.dma_start(
                        maskT[:], maskT_d.ap().rearrange("(kb p) q -> p kb q", p=128).bitcast(F32R))
                    attention(kT, v_aug, qT, att_ps, att_pso, ppool,
                              lambda h, kb: maskT[:, kb, :], oT_r)

            project(w_d["wo_s"], [oT_r],
                    lambda dout, ci, ps: nc.vector.tensor_add(
                        out=ax[:, dout, :], in0=ps[:], in1=xq[:, dout, :]))

        # ================= Phase D: cross-attention block =================
        de = top.enter_context(ExitStack())
        cxp = de.enter_context(tc.tile_pool(name="cxp", bufs=1))
        cx = cxp.tile([128, NB, TQ], F32)
        with ExitStack() as dd:
            with ExitStack() as att_scope:
                with tc.tile_pool(name="h2p", bufs=1) as h2p:
                    h2 = h2p.tile([128, NB, TQ], F32R)
                    rmsnorm(ax, g_ca, h2)
                    catt = att_scope.enter_context(tc.tile_pool(name="catt", bufs=1, side="right"))
                    qcT = catt.tile([128, NB, TQ], F32R, tag="qcT")
                    kcT = catt.tile([128, NB, LS], F32R, tag="kcT")
                    vc_aug = catt.tile([128, NKB, H * 65], F32R, tag="vc_aug")
                    nc.vector.memset(
                        vc_aug[:].rearrange("p kb (h x) -> p kb h x", x=65)[:, :, :, 64:65].bitcast(F32), 1.0)
                    project(w_d["wq_c"], [h2],
                            lambda dout, ci, ps: nc.scalar.mul(out=qcT[:, dout, :], in_=ps[:], mul=0.125))

                with tc.tile_pool(name="memp", bufs=1) as memp:
                    mq = memp.tile([128, NB, TQ], F32R, tag="mq")
                    mo = memp.tile([128, NB, TQ], F32R, tag="mo")
                    nc.sync.dma_start(mq[:], mq_d.ap().rearrange("(o p) t -> p o t", p=128).bitcast(F32R))
                    nc.sync.dma_start(mo[:], mo_d.ap().rearrange("(o p) t -> p o t", p=128).bitcast(F32R))
                    project(w_d["wk_c"], [mq, mo],
                            lambda dout, ci, ps: nc.scalar.copy(
                                out=kcT[:, dout, ci * TQ:(ci + 1) * TQ], in_=ps[:]))
                    project_tokmajor(w_d["wv_c"], [mq, mo], vc_aug)

                nc.sync.dma_start(kc_d.ap().rearrange("(o p) t -> p o t", p=128),
                                  kcT[:, :, 0:TQ].bitcast(F32))
                for tb in range(4):
                    nc.sync.dma_start(
                        vc_d.ap()[tb * 128:(tb + 1) * 128, :].rearrange(
                            "p (h hd) -> p h hd", hd=64),
                        vc_aug[:, tb, :].rearrange("p (h x) -> p h x", x=65)[:, :, 0:64].bitcast(F32))

                oTc_r = dd.enter_context(tc.tile_pool(name="oTcrp", bufs=1)).tile(
                    [128, NB, TQ], F32R, name="oTc_r")
                with tc.tile_pool(name="att_ps", bufs=3, space="PSUM") as att_ps, \
                     tc.tile_pool(name="att_pso", bufs=2, space="PSUM") as att_pso, \
                     tc.tile_pool(name="ppool", bufs=2) as ppool, \
                     tc.tile_pool(name="pepool", bufs=4) as pepool:
                    def pe_bias(h, kb):
                        t = pepool.tile([128, TQ], F32R, tag="pe")
                        nc.sync.dma_start(
                            t[:], peT_d.ap()[h, kb * 128:(kb + 1) * 128, :].bitcast(F32R))
                        return t[:]
                    attention(kcT, vc_aug, qcT, att_ps, att_pso, ppool, pe_bias, oTc_r)

            project(w_d["wo_c"], [oTc_r],
                    lambda dout, ci, ps: nc.vector.tensor_add(
                        out=cx[:, dout, :], in0=ps[:], in1=ax[:, dout, :]))

        # ================= Phase E: FFN =================
        with ExitStack() as ef:
            ffn = ef.enter_context(tc.tile_pool(name="ffn", bufs=1))
            h3 = ffn.tile([128, NB, TQ], F32R, tag="h3")
            rmsnorm(cx, g_m, h3)
            relu = ffn.tile([128, NFB, TQ], F32R, tag="relu")
            for fs in range(8):
                w_sb = wpool.tile([128, NB, 512], F32R, tag="wslab")
                nc.sync.dma_start(
                    w_sb[:], w1_d.ap().rearrange("(o p) f -> p o f", p=128)
                    [:, :, fs * 512:(fs + 1) * 512].bitcast(F32R))
                for d4 in range(4):
                    ps = pspool.tile([128, TQ], F32, tag="proj_ps")
                    for din in range(NB):
                        nc.tensor.matmul(ps[:], w_sb[:, din, d4 * 128:(d4 + 1) * 128],
                                         h3[:, din, :], start=(din == 0), stop=(din == NB - 1))
                    nc.scalar.activation(out=relu[:, fs * 4 + d4, :], in_=ps[:], func=Act.Relu)

            mlp_sb = ffn.tile([128, NB, TQ], F32, tag="mlp")
            with tc.tile_pool(name="ffn_ps", bufs=1, space="PSUM") as ffn_ps:
                for g in range(2):
                    psums = [ffn_ps.tile([128, TQ], F32, tag=f"acc{d4}", name=f"acc{d4}") for d4 in range(4)]
                    for fs in range(8):
                        w2_sb = wpool.tile([128, 4, 512], F32R, tag="wslab")
                        nc.sync.dma_start(
                            w2_sb[:], w2_d.ap()[fs * 512:(fs + 1) * 512, g * 512:(g + 1) * 512]
                            .rearrange("(o p) d -> p o d", p=128).bitcast(F32R))
                        for d4 in range(4):
                            for fsub in range(4):
                                nc.tensor.matmul(
                                    psums[d4][:], w2_sb[:, fsub, d4 * 128:(d4 + 1) * 128],
                                    relu[:, fs * 4 + fsub, :],
                                    start=(fs == 0 and fsub == 0),
                                    stop=(fs == 7 and fsub == 3))
                    for d4 in range(4):
                        dout = g * 4 + d4
                        nc.vector.tensor_add(out=mlp_sb[:, dout, :], in0=psums[d4][:],
                                             in1=cx[:, dout, :])
            nc.sync.dma_start(mlp_d.ap().rearrange("(o p) t -> p o t", p=128), mlp_sb[:])

    nc.finalize()
    return nc


_NC = None


def _get_nc():
    global _NC
    if _NC is None:
        _NC = _build()
    return _NC


def _make_in_maps(x, memory, position_embedding, causal_mask,
                  gamma_sa, wq_s, wk_s, wv_s, wo_s,
                  gamma_ca, wq_c, wk_c, wv_c, wo_c,
                  gamma_m, w1, w2):
    C = np.ascontiguousarray
    halves = [slice(0, TQ), slice(TQ, LT)]
    g = lambda v: C(v.reshape(NB, 128).T)
    shared = {
        "g_sa": g(gamma_sa), "g_ca": g(gamma_ca), "g_m": g(gamma_m),
        "wq_s": wq_s, "wk_s": wk_s, "wv_s": wv_s, "wo_s": wo_s,
        "wq_c": wq_c, "wk_c": wk_c, "wv_c": wv_c, "wo_c": wo_c,
        "w1": w1, "w2": w2,
    }
    # mask/pe depend only on the query half: precompute both variants
    ct = causal_mask.T  # [k, q]
    pe = position_embedding[0]  # [H, LT(q), LS(k)]
    mask_v, pe_v = [], []
    for half in range(2):
        own, oth = halves[half], halves[1 - half]
        A = ct[:, own]
        mask_v.append(C(np.concatenate([A[own], A[oth]], axis=0)))
        PT = pe[:, own, :].transpose(0, 2, 1)  # [H, LS(k), TQ]
        pe_v.append(C(np.concatenate([PT[:, own, :], PT[:, oth, :]], axis=1)))
    in_maps = []
    for core in range(8):
        b, half = core // 2, core % 2
        own, oth = halves[half], halves[1 - half]
        xT = x[b].T
        mT = memory[b].T
        m = dict(shared)
        m.update({
            "xq": C(xT[:, own]), "xo": C(xT[:, oth]),
            "mq": C(mT[:, own]), "mo": C(mT[:, oth]),
            "maskT": mask_v[half], "peT": pe_v[half],
        })
        in_maps.append(m)
    return in_maps


def _assemble(results):
    halves = [slice(0, TQ), slice(TQ, LT)]
    mlp = np.empty((B, LT, D), np.float32)
    k_s = np.empty((B, H, LT, HD), np.float32)
    v_s = np.empty((B, H, LT, HD), np.float32)
    k_c = np.empty((B, H, LS, HD), np.float32)
    v_c = np.empty((B, H, LS, HD), np.float32)
    for core in range(8):
        b, half = core // 2, core % 2
        own = halves[half]
        r = results[core]
        mlp[b, own, :] = r["mlpT"].T
        k_s[b, :, own, :] = r["ks_own"].reshape(H, HD, TQ).transpose(0, 2, 1)
        v_s[b, :, own, :] = r["vs_own"].reshape(TQ, H, HD).transpose(1, 0, 2)
        k_c[b, :, own, :] = r["kc_own"].reshape(H, HD, TQ).transpose(0, 2, 1)
        v_c[b, :, own, :] = r["vc_own"].reshape(TQ, H, HD).transpose(1, 0, 2)
    return mlp, (k_s, v_s), (k_c, v_c)


def kernel(**inputs):
    inputs = {k: np.asarray(v, dtype=np.float32) for k, v in inputs.items()}
    in_maps = _make_in_maps(**inputs)
    nc = _get_nc()
    res = run_bass_kernel_spmd(nc, in_maps, core_ids=list(range(8)))
    return _assemble(res.results)


# revision 11
# speedup vs baseline: 2.8505x; 2.8505x over previous
"""Trainium2 Bass kernel for a transformer decoder layer (self-attn + cross-attn + FFN).

Sharding: 8 cores = 4 batches x 2 query-halves. Zero collectives: each core
computes the full layer for its 512 query tokens (feature-major layout), and
recomputes the full-sequence K/V for its batch (cheap relative to total work).
Causality is handled purely by mask data: each core sees K/V token chunks in
core-local order (own half first), so one SPMD program serves all cores.

All heavy matmuls run as float32r (full PE rate, ~1e-4 relative error).
Softmax: no max-subtraction (scores are O(6); exp is safe in fp32); the
denominator comes free from a ones-column appended to V in the PV matmul.
"""
import sys
sys.path.insert(0, '/opt/trn_rl_repo')

import numpy as np

import concourse.bass as bass
from concourse import bacc
import concourse.mybir as mybir
import concourse.tile as tile
from concourse.bass_utils import run_bass_kernel_spmd
from concourse.masks import make_identity
from contextlib import ExitStack

F32 = mybir.dt.float32
F32R = mybir.dt.float32r
Act = mybir.ActivationFunctionType
Alu = mybir.AluOpType

B, LT, LS, D, H, HD, F = 4, 1024, 1024, 1024, 16, 64, 4096
TQ = LT // 2          # tokens per core (query half)
NB = D // 128         # feature blocks (8)
NKB = LT // 128       # key blocks (8)
NFB = F // 128        # FFN feature blocks (32)
EPS = 1e-6


def _build(loop_n=None):
    nc = bacc.Bacc("TRN2", target_bir_lowering=False)

    di = lambda n, shp: nc.dram_tensor(n, shp, F32, kind="ExternalInput")
    do = lambda n, shp: nc.dram_tensor(n, shp, F32, kind="ExternalOutput")

    xq_d = di("xq", [D, TQ])            # own x chunk, feature-major
    xo_d = di("xo", [D, TQ])            # other x chunk
    mq_d = di("mq", [D, TQ])            # own memory chunk
    mo_d = di("mo", [D, TQ])            # other memory chunk
    maskT_d = di("maskT", [LT, TQ])     # causal mask [k(core order), q]
    peT_d = di("peT", [H, LS, TQ])      # position bias [h, k(core order), q]
    g_sa_d = di("g_sa", [128, NB])
    g_ca_d = di("g_ca", [128, NB])
    g_m_d = di("g_m", [128, NB])
    w_d = {n: di(n, [D, D]) for n in
           ("wq_s", "wk_s", "wv_s", "wo_s", "wq_c", "wk_c", "wv_c", "wo_c")}
    w1_d = di("w1", [D, F])
    w2_d = di("w2", [F, D])

    mlp_d = do("mlpT", [D, TQ])
    ks_d = do("ks_own", [D, TQ])
    vs_d = do("vs_own", [TQ, D])
    kc_d = do("kc_own", [D, TQ])
    vc_d = do("vc_own", [TQ, D])

    def w_slab_ap(w, lo, n):
        # [D, D] -> [128, NB, n] slice of columns [lo, lo+n)
        return w.ap().rearrange("(o p) d -> p o d", p=128)[:, :, lo:lo + n].bitcast(F32R)

    with tile.TileContext(nc) as tc, ExitStack() as top:
        const = top.enter_context(tc.tile_pool(name="const", bufs=1))
        tiny = top.enter_context(tc.tile_pool(name="tiny", bufs=2))
        wpool = top.enter_context(tc.tile_pool(name="wpool", bufs=2))
        pspool = top.enter_context(tc.tile_pool(name="pspool", bufs=2, space="PSUM"))
        sqpool = top.enter_context(tc.tile_pool(name="sqpool", bufs=2))
        norm_ps = top.enter_context(tc.tile_pool(name="norm_ps", bufs=1, space="PSUM"))
        bcpool = top.enter_context(tc.tile_pool(name="bcpool", bufs=2))
        axp = top.enter_context(tc.tile_pool(name="axp", bufs=1))

        ident_f = const.tile([128, 128], F32)
        make_identity(nc, ident_f[:])
        ident = const.tile([128, 128], F32R)
        nc.vector.tensor_copy(out=ident[:], in_=ident_f[:])
        ones_f = const.tile([128, 1], F32)
        nc.vector.memset(ones_f[:], 1.0)
        ones_r = const.tile([128, 1], F32R)
        nc.vector.tensor_copy(out=ones_r[:], in_=ones_f[:])
        eps_t = const.tile([1, 1], F32)
        nc.vector.memset(eps_t[:], EPS)
        g_sa = const.tile([128, NB], F32)
        g_ca = const.tile([128, NB], F32)
        g_m = const.tile([128, NB], F32)
        nc.sync.dma_start(g_sa[:], g_sa_d.ap())
        nc.sync.dma_start(g_ca[:], g_ca_d.ap())
        nc.sync.dma_start(g_m[:], g_m_d.ap())

        def rmsnorm(x_t, gamma, out_t):
            """x_t [128, NB, TQ] F32 -> out_t [128, NB, TQ] F32R (x * rstd * gamma)."""
            ps = norm_ps.tile([1, TQ], F32, tag="norm_ps")
            for b in range(NB):
                sq = sqpool.tile([128, TQ], F32R, tag="sq")
                nc.vector.tensor_mul(out=sq[:], in0=x_t[:, b, :], in1=x_t[:, b, :])
                nc.tensor.matmul(ps[:], ones_r[:], sq[:],
                                 start=(b == 0), stop=(b == NB - 1))
            rstd = tiny.tile([1, TQ], F32, tag="rstd")
            nc.scalar.activation(out=rstd[:], in_=ps[:], func=Act.Sqrt,
                                 bias=eps_t[:], scale=1.0 / D)
            nc.vector.reciprocal(out=rstd[:], in_=rstd[:])
            rstd_b = tiny.tile([128, TQ], F32, tag="rstd_b")
            nc.gpsimd.partition_broadcast(rstd_b[:], rstd[:])
            for b in range(NB):
                nc.vector.scalar_tensor_tensor(
                    out=out_t[:, b, :], in0=x_t[:, b, :], scalar=gamma[:, b:b + 1],
                    in1=rstd_b[:], op0=Alu.mult, op1=Alu.mult)

        def project(w, rhs_list, consumer):
            """out[dout, t] = sum_din w[din, dout] * rhs[din, t].
            rhs_list: list of [128, NB, TQ] F32R tiles (column chunks).
            consumer(dout_blk, chunk_idx, psum[128, TQ])."""
            for so in range(2):
                w_sb = wpool.tile([128, NB, 512], F32R, tag="wslab")
                nc.sync.dma_start(w_sb[:], w_slab_ap(w, so * 512, 512))
                for d4 in range(4):
                    dout = so * 4 + d4
                    for ci, rhs in enumerate(rhs_list):
                        ps = pspool.tile([128, TQ], F32, tag="proj_ps")
                        for din in range(NB):
                            nc.tensor.matmul(
                                ps[:], w_sb[:, din, d4 * 128:(d4 + 1) * 128],
                                rhs[:, din, :],
                                start=(din == 0), stop=(din == NB - 1))
                        consumer(dout, ci, ps)

        def project_tokmajor(w, rhs_list, v_aug):
            """v[t, dout] token-major into v_aug [128, NKB, H*65] (65-col head groups)."""
            for half in range(2):
                w_sb = wpool.tile([128, NB, 512], F32R, tag="wslab")
                nc.sync.dma_start(w_sb[:], w_slab_ap(w, half * 512, 512))
                for ci, rhs in enumerate(rhs_list):
                    for tb in range(4):
                        ps = pspool.tile([128, TQ], F32, tag="proj_ps")
                        for din in range(NB):
                            nc.tensor.matmul(
                                ps[:], rhs[:, din, tb * 128:(tb + 1) * 128],
                                w_sb[:, din, :],
                                start=(din == 0), stop=(din == NB - 1))
                        dst = v_aug[:, ci * 4 + tb, :].rearrange(
                            "p (h x) -> p h x", x=65)[:, half * 8:(half + 1) * 8, 0:64]
                        nc.scalar.copy(out=dst, in_=ps[:].rearrange("p (h hd) -> p h hd", hd=64))

        def attention(kT, v_aug, qT, att_ps, att_pso, ppool, get_bias, oT_r):
            """Attention with fused per-head softmax normalization -> oT_r [128, NB, TQ] F32R."""
            for h in range(H):
                po = 64 * (h % 2)
                db = h // 2
                psum_o = att_pso.tile([65, TQ], F32, tag="psum_o")
                for kb in range(NKB):
                    psum_s = att_ps.tile([128, TQ], F32, tag="psum_s")
                    nc.tensor.matmul(psum_s[:],
                                     kT[po:po + 64, db, kb * 128:(kb + 1) * 128],
                                     qT[po:po + 64, db, :], start=True, stop=False)
                    nc.tensor.matmul(psum_s[:], ident[:], get_bias(h, kb),
                                     start=False, stop=True)
                    p_sb = ppool.tile([128, TQ], F32R, tag="p")
                    nc.scalar.activation(out=p_sb[:], in_=psum_s[:], func=Act.Exp)
                    nc.tensor.matmul(psum_o[:], v_aug[:, kb, h * 65:(h + 1) * 65],
                                     p_sb[:], start=(kb == 0), stop=(kb == NKB - 1))
                rsum = tiny.tile([1, TQ], F32, tag="rsum")
                nc.vector.reciprocal(out=rsum[:], in_=psum_o[64:65, :])
                bc = bcpool.tile([128, TQ], F32, tag="bc")
                nc.gpsimd.partition_broadcast(bc[:], rsum[:])
                nc.vector.tensor_mul(out=oT_r[po:po + 64, db, :],
                                     in0=psum_o[0:64, :], in1=bc[0:64, :])

        ax = axp.tile([128, NB, TQ], F32)

        loop_ctx = tc.For_i(0, loop_n, 1) if loop_n else None
        if loop_ctx is not None:
            loop_ctx.__enter__()

        # ================= Phases A-C: self-attention block =================
        with ExitStack() as abc:
            xq = abc.enter_context(tc.tile_pool(name="xqp", bufs=1)).tile([128, NB, TQ], F32, name="xq")
            nc.sync.dma_start(xq[:], xq_d.ap().rearrange("(o p) t -> p o t", p=128))
            with ExitStack() as att_scope:
                with ExitStack() as hs:
                    hp = hs.enter_context(tc.tile_pool(name="hpool", bufs=1))
                    h0 = hp.tile([128, NB, TQ], F32R, tag="h0")
                    h1 = hp.tile([128, NB, TQ], F32R, tag="h1")
                    with tc.tile_pool(name="xop", bufs=1) as xop:
                        xo = xop.tile([128, NB, TQ], F32)
                        nc.sync.dma_start(xo[:], xo_d.ap().rearrange("(o p) t -> p o t", p=128))
                        rmsnorm(xq, g_sa, h0)
                        rmsnorm(xo, g_sa, h1)
                    atti = att_scope.enter_context(tc.tile_pool(name="att_in", bufs=1, side="right"))
                    qT = atti.tile([128, NB, TQ], F32R, tag="qT")
                    kT = atti.tile([128, NB, LT], F32R, tag="kT")
                    v_aug = atti.tile([128, NKB, H * 65], F32R, tag="v_aug")
                    nc.vector.memset(
                        v_aug[:].rearrange("p kb (h x) -> p kb h x", x=65)[:, :, :, 64:65].bitcast(F32), 1.0)
                    project(w_d["wq_s"], [h0],
                            lambda dout, ci, ps: nc.scalar.mul(out=qT[:, dout, :], in_=ps[:], mul=0.125))
                    project(w_d["wk_s"], [h0, h1],
                            lambda dout, ci, ps: nc.scalar.copy(
                                out=kT[:, dout, ci * TQ:(ci + 1) * TQ], in_=ps[:]))
                    project_tokmajor(w_d["wv_s"], [h0, h1], v_aug)

                # K/V outputs (own chunk = chunk 0)
                nc.sync.dma_start(ks_d.ap().rearrange("(o p) t -> p o t", p=128),
                                  kT[:, :, 0:TQ].bitcast(F32))
                for tb in range(4):
                    nc.sync.dma_start(
                        vs_d.ap()[tb * 128:(tb + 1) * 128, :].rearrange(
                            "p (h hd) -> p h hd", hd=64),
                        v_aug[:, tb, :].rearrange("p (h x) -> p h x", x=65)[:, :, 0:64].bitcast(F32))

                oT_r = abc.enter_context(tc.tile_pool(name="oTrp", bufs=1)).tile(
                    [128, NB, TQ], F32R, name="oT_r")
                with tc.tile_pool(name="maskp", bufs=1) as maskp, \
                     tc.tile_pool(name="att_ps", bufs=3, space="PSUM") as att_ps, \
                     tc.tile_pool(name="att_pso", bufs=2, space="PSUM") as att_pso, \
                     tc.tile_pool(name="ppool", bufs=2) as ppool:
                    maskT = maskp.tile([128, NKB, TQ], F32R)
                    nc.sync.dma_start(
                        maskT[:], maskT_d.ap().rearrange("(kb p) q -> p kb q", p=128).bitcast(F32R))
                    attention(kT, v_aug, qT, att_ps, att_pso, ppool,
                              lambda h, kb: maskT[:, kb, :], oT_r)

            project(w_d["wo_s"], [oT_r],
                    lambda dout, ci, ps: nc.vector.tensor_add(
                        out=ax[:, dout, :], in0=ps[:], in1=xq[:, dout, :]))

        # ================= Phase D: cross-attention block =================
        de = top.enter_context(ExitStack())
        cxp = de.enter_context(tc.tile_pool(name="cxp", bufs=1))
        cx = cxp.tile([128, NB, TQ], F32)
        with ExitStack() as dd:
            with ExitStack() as att_scope:
                with tc.tile_pool(name="h2p", bufs=1) as h2p:
                    h2 = h2p.tile([128, NB, TQ], F32R)
                    rmsnorm(ax, g_ca, h2)
                    catt = att_scope.enter_context(tc.tile_pool(name="catt", bufs=1, side="right"))
                    qcT = catt.tile([128, NB, TQ], F32R, tag="qcT")
                    kcT = catt.tile([128, NB, LS], F32R, tag="kcT")
                    vc_aug = catt.tile([128, NKB, H * 65], F32R, tag="vc_aug")
                    nc.vector.memset(
                        vc_aug[:].rearrange("p kb (h x) -> p kb h x", x=65)[:, :, :, 64:65].bitcast(F32), 1.0)
                    project(w_d["wq_c"], [h2],
                            lambda dout, ci, ps: nc.scalar.mul(out=qcT[:, dout, :], in_=ps[:], mul=0.125))

                with tc.tile_pool(name="memp", bufs=1) as memp:
                    mq = memp.tile([128, NB, TQ], F32R, tag="mq")
                    mo = memp.tile([128, NB, TQ], F32R, tag="mo")
                    nc.sync.dma_start(mq[:], mq_d.ap().rearrange("(o p) t -> p o t", p=128).bitcast(F32R))
                    nc.sync.dma_start(mo[:], mo_d.ap().rearrange("(o p) t -> p o t", p=128).bitcast(F32R))
                    project(w_d["wk_c"], [mq, mo],
                            lambda dout, ci, ps: nc.scalar.copy(
                                out=kcT[:, dout, ci * TQ:(ci + 1) * TQ], in_=ps[:]))
                    project_tokmajor(w_d["wv_c"], [mq, mo], vc_aug)

                nc.sync.dma_start(kc_d.ap().rearrange("(o p) t -> p o t", p=128),
                                  kcT[:, :, 0:TQ].bitcast(F32))
                for tb in range(4):
                    nc.sync.dma_start(
                        vc_d.ap()[tb * 128:(tb + 1) * 128, :].rearrange(
                            "p (h hd) -> p h hd", hd=64),
                        vc_aug[:, tb, :].rearrange("p (h x) -> p h x", x=65)[:, :, 0:64].bitcast(F32))

                oTc_r = dd.enter_context(tc.tile_pool(name="oTcrp", bufs=1)).tile(
                    [128, NB, TQ], F32R, name="oTc_r")
                with tc.tile_pool(name="att_ps", bufs=3, space="PSUM") as att_ps, \
                     tc.tile_pool(name="att_pso", bufs=2, space="PSUM") as att_pso, \
                     tc.tile_pool(name="ppool", bufs=2) as ppool, \
                     tc.tile_pool(name="pepool", bufs=4) as pepool:
                    def pe_bias(h, kb):
                        t = pepool.tile([128, TQ], F32R, tag="pe")
                        nc.sync.dma_start(
                            t[:], peT_d.ap()[h, kb * 128:(kb + 1) * 128, :].bitcast(F32R))
                        return t[:]
                    attention(kcT, vc_aug, qcT, att_ps, att_pso, ppool, pe_bias, oTc_r)

            project(w_d["wo_c"], [oTc_r],
                    lambda dout, ci, ps: nc.vector.tensor_add(
                        out=cx[:, dout, :], in0=ps[:], in1=ax[:, dout, :]))

        # ================= Phase E: FFN =================
        with ExitStack() as ef:
            ffn = ef.enter_context(tc.tile_pool(name="ffn", bufs=1))
            h3 = ffn.tile([128, NB, TQ], F32R, tag="h3")
            rmsnorm(cx, g_m, h3)
            relu = ffn.tile([128, NFB, TQ], F32R, tag="relu")
            for fs in range(8):
                w_sb = wpool.tile([128, NB, 512], F32R, tag="wslab")
                nc.sync.dma_start(
                    w_sb[:], w1_d.ap().rearrange("(o p) f -> p o f", p=128)
                    [:, :, fs * 512:(fs + 1) * 512].bitcast(F32R))
                for d4 in range(4):
                    ps = pspool.tile([128, TQ], F32, tag="proj_ps")
                    for din in range(NB):
                        nc.tensor.matmul(ps[:], w_sb[:, din, d4 * 128:(d4 + 1) * 128],
                                         h3[:, din, :], start=(din == 0), stop=(din == NB - 1))
                    nc.scalar.activation(out=relu[:, fs * 4 + d4, :], in_=ps[:], func=Act.Relu)

            mlp_sb = ffn.tile([128, NB, TQ], F32, tag="mlp")
            with tc.tile_pool(name="ffn_ps", bufs=1, space="PSUM") as ffn_ps:
                for g in range(2):
                    psums = [ffn_ps.tile([128, TQ], F32, tag=f"acc{d4}", name=f"acc{d4}") for d4 in range(4)]
                    for fs in range(8):
                        w2_sb = wpool.tile([128, 4, 512], F32R, tag="wslab")
                        nc.sync.dma_start(
                            w2_sb[:], w2_d.ap()[fs * 512:(fs + 1) * 512, g * 512:(g + 1) * 512]
                            .rearrange("(o p) d -> p o d", p=128).bitcast(F32R))
                        for d4 in range(4):
                            for fsub in range(4):
                                nc.tensor.matmul(
                                    psums[d4][:], w2_sb[:, fsub, d4 * 128:(d4 + 1) * 128],
                                    relu[:, fs * 4 + fsub, :],
                                    start=(fs == 0 and fsub == 0),
                                    stop=(fs == 7 and fsub == 3))
                    for d4 in range(4):
                        dout = g * 4 + d4
                        nc.vector.tensor_add(out=mlp_sb[:, dout, :], in0=psums[d4][:],
                                             in1=cx[:, dout, :])
            nc.sync.dma_start(mlp_d.ap().rearrange("(o p) t -> p o t", p=128), mlp_sb[:])

        if loop_ctx is not None:
            loop_ctx.__exit__(None, None, None)

    nc.finalize()
    return nc


_NC = None


def _get_nc():
    global _NC
    if _NC is None:
        _NC = _build()
    return _NC


def _make_in_maps(x, memory, position_embedding, causal_mask,
                  gamma_sa, wq_s, wk_s, wv_s, wo_s,
                  gamma_ca, wq_c, wk_c, wv_c, wo_c,
                  gamma_m, w1, w2):
    C = np.ascontiguousarray
    halves = [slice(0, TQ), slice(TQ, LT)]
    g = lambda v: C(v.reshape(NB, 128).T)
    shared = {
        "g_sa": g(gamma_sa), "g_ca": g(gamma_ca), "g_m": g(gamma_m),
        "wq_s": wq_s, "wk_s": wk_s, "wv_s": wv_s, "wo_s": wo_s,
        "wq_c": wq_c, "wk_c": wk_c, "wv_c": wv_c, "wo_c": wo_c,
        "w1": w1, "w2": w2,
    }
    # mask/pe depend only on the query half: precompute both variants
    ct = causal_mask.T  # [k, q]
    pe = position_embedding[0]  # [H, LT(q), LS(k)]
    mask_v, pe_v = [], []
    for half in range(2):
        own, oth = halves[half], halves[1 - half]
        A = ct[:, own]
        mask_v.append(C(np.concatenate([A[own], A[oth]], axis=0)))
        PT = pe[:, own, :].transpose(0, 2, 1)  # [H, LS(k), TQ]
        pe_v.append(C(np.concatenate([PT[:, own, :], PT[:, oth, :]], axis=1)))
    in_maps = []
    for core in range(8):
        b, half = core // 2, core % 2
        own, oth = halves[half], halves[1 - half]
        xT = x[b].T
        mT = memory[b].T
        m = dict(shared)
        m.update({
            "xq": C(xT[:, own]), "xo": C(xT[:, oth]),
            "mq": C(mT[:, own]), "mo": C(mT[:, oth]),
            "maskT": mask_v[half], "peT": pe_v[half],
        })
        in_maps.append(m)
    return in_maps


def _assemble(results):
    halves = [slice(0, TQ), slice(TQ, LT)]
    mlp = np.empty((B, LT, D), np.float32)
    k_s = np.empty((B, H, LT, HD), np.float32)
    v_s = np.empty((B, H, LT, HD), np.float32)
    k_c = np.empty((B, H, LS, HD), np.float32)
    v_c = np.empty((B, H, LS, HD), np.float32)
    for core in range(8):
        b, half = core // 2, core % 2
        own = halves[half]
        r = results[core]
        mlp[b, own, :] = r["mlpT"].T
        k_s[b, :, own, :] = r["ks_own"].reshape(H, HD, TQ).transpose(0, 2, 1)
        v_s[b, :, own, :] = r["vs_own"].reshape(TQ, H, HD).transpose(1, 0, 2)
        k_c[b, :, own, :] = r["kc_own"].reshape(H, HD, TQ).transpose(0, 2, 1)
        v_c[b, :, own, :] = r["vc_own"].reshape(TQ, H, HD).transpose(1, 0, 2)
    return mlp, (k_s, v_s), (k_c, v_c)


def kernel(**inputs):
    inputs = {k: np.asarray(v, dtype=np.float32) for k, v in inputs.items()}
    in_maps = _make_in_maps(**inputs)
    nc = _get_nc()
    res = run_bass_kernel_spmd(nc, in_maps, core_ids=list(range(8)))
    return _assemble(res.results)
